# revision 1
# baseline (speedup 1.0000x reference)
"""MoE gate (group-limited greedy routing) on 8 Trainium2 NeuronCores.

Math (per token t):
    logits = x[t, 1:] @ weight.T                    (64 experts)
    scores = sigmoid(logits)
    sb     = scores + bias
    group_scores[g] = sum(top2(sb[g*8:(g+1)*8]))    (8 groups)
    keep top-4 groups; mask the rest to -inf
    top-8 experts of masked sb -> indices
    weights = 2.5 * normalize(scores[indices])

Device strategy per core (4096 tokens), v2:
  - x streamed at 3 bytes/elem: fp16 hi + e3m4 fp8 of the scaled residual
    (res * 2^12), packed per (k-tile, chunk) into one u8 DMA of 1536B/row.
    W is fp16 hi/lo packed [wh|wl] (fp32-exact) + e3m4 w8 (w * 2^7) for the
    residual stream.  logits = (PA_hi + PA_lo) + 2^-19 * PB.
  - fp32 transpose (PE identity matmul) to token-major, sigmoid on ACT.
  - top-k batched over 1024 tokens (NB=8 blocks) to amortize per-op
    overhead; elementwise ops split across Pool (nc.gpsimd) and DVE.
"""

import sys

sys.path.insert(0, "/opt/trn_rl_repo")

import numpy as np
import ml_dtypes
import concourse.bacc as bacc
import concourse.mybir as mybir
from concourse.tile import TileContext
from concourse.bass_utils import run_bass_kernel_spmd

F32 = mybir.dt.float32
F16 = mybir.dt.float16
F8 = mybir.dt.float8e3
U8 = mybir.dt.uint8
U32 = mybir.dt.uint32
I32 = mybir.dt.int32
Alu = mybir.AluOpType
Act = mybir.ActivationFunctionType
AxX = mybir.AxisListType.X

T = 32768
DIM = 2048
E = 64
G = 8
GS = E // G          # 8 experts per group
TOPK = 8
ROUTE_SCALE = 2.5

NCORES = 8
TPC = T // NCORES    # 4096 tokens per core
CHUNK = 512          # tokens per matmul chunk
NCHUNK = TPC // CHUNK
KP = 128             # contraction tile
KT = DIM // KP       # 16 k-tiles (feature dim padded 2047 -> 2048)

K8 = 4096.0          # 2^12: residual scale into e3m4 range (max ~11 < 15.5)
KW8 = 128.0          # 2^7:  w scale into e3m4 range (max ~6.8)
C8 = 1.0 / (K8 * KW8)  # 2^-19

NEG = -1.0e9

_CACHE = {}


def _topk_batch(nc, pool, sc, br8, wi_out, t0, cfg, NB):
    """Group-limited top-8 for one [128, NB, 64] blocked score batch."""
    P = 128
    V = nc.vector
    PL = nc.gpsimd      # Pool engine

    def pick(key, default):
        return PL if cfg.get(key, default) == "pool" else V

    e_sb = pick("sb_eng", "pool")
    e_eq = V
    e_wo = pick("wo_eng", "pool")

    def t4(ap):  # [P, NB, G, GS] view
        return ap.rearrange("p b (g s) -> p b g s", s=GS)

    sb = pool.tile([P, NB, E], F32, tag="sb")
    e_sb.tensor_add(sb[:], sc[:], br8[:, 0:NB, :])

    # group top-2 sum: m1 = group max, m2 = max after masking m1 out
    m1 = pool.tile([P, NB, G], F32, tag="m1")
    V.tensor_reduce(m1[:], t4(sb[:]), axis=AxX, op=Alu.max)
    eq = pool.tile([P, NB, E], F32, tag="eqg")
    V.tensor_tensor(t4(eq[:]), t4(sb[:]),
                       m1[:].unsqueeze(3).to_broadcast([P, NB, G, GS]),
                       op=Alu.is_equal)
    sb2 = pool.tile([P, NB, E], F32, tag="sb2")
    V.scalar_tensor_tensor(out=sb2[:], in0=eq[:], scalar=NEG, in1=sb[:],
                           op0=Alu.mult, op1=Alu.add)
    m2 = pool.tile([P, NB, G], F32, tag="m2")
    V.tensor_reduce(m2[:], t4(sb2[:]), axis=AxX, op=Alu.max)
    gs_t = pool.tile([P, NB, G], F32, tag="gs")
    PL.tensor_add(gs_t[:], m1[:], m2[:])

    # per-token threshold tau = 4th largest group score
    g8 = pool.tile([P, NB, 8], F32, tag="g8")
    for b in range(NB):
        V.max(out=g8[:, b, :], in_=gs_t[:, b, :])
    pen = pool.tile([P, NB, G], F32, tag="pen")
    V.tensor_tensor(pen[:], gs_t[:],
                    g8[:, :, 3:4].to_broadcast([P, NB, G]), op=Alu.is_lt)
    mk = pool.tile([P, NB, E], F32, tag="mk")
    V.scalar_tensor_tensor(
        out=t4(mk[:]),
        in0=pen[:].unsqueeze(3).to_broadcast([P, NB, G, GS]),
        scalar=NEG, in1=t4(sb[:]), op0=Alu.mult, op1=Alu.add)

    # per-token top-8 (sorted values + indices); wi packs [weights | indices]
    v8 = pool.tile([P, NB, 8], F32, tag="v8")
    wi = pool.tile([P, NB, 2 * TOPK], F32, tag="wi")
    ix = wi[:, :, TOPK:2 * TOPK].bitcast(U32)
    for b in range(NB):
        V.max(out=v8[:, b, :], in_=mk[:, b, :])
        V.max_index(out=ix[:, b, :], in_max=v8[:, b, :], in_values=mk[:, b, :])

    # ordered gather of original scores: (mk == v8_j) * sc, summed over e.
    # Ranks [0, GD): fused stt-accum per (block, rank) on DVE (Pool cannot
    # run TensorScalarPtr).  Ranks [GD, 8): tensor_tensor pair on Pool with
    # a DVE reduce.
    gat = pool.tile([P, NB, TOPK], F32, tag="gat")
    junk = pool.tile([P, E], F32, tag="junk")
    GD = cfg.get("gd", 0)
    JP = TOPK - GD
    for b in range(NB):
        for j in range(GD):
            V.scalar_tensor_tensor(
                out=junk[:],
                in0=mk[:, b, :], scalar=v8[:, b, j:j + 1], in1=sc[:, b, :],
                op0=Alu.is_equal, op1=Alu.mult, accum_out=gat[:, b, j:j + 1])
    if JP > 0:
        # remaining ranks via wide compare on DVE + Pool multiply + DVE reduce
        eq3 = pool.tile([P, NB, JP, E], F32, tag="eq3")
        V.tensor_tensor(
            eq3[:], mk[:].unsqueeze(2).to_broadcast([P, NB, JP, E]),
            v8[:, :, GD:TOPK].unsqueeze(3).to_broadcast([P, NB, JP, E]),
            op=Alu.is_equal)
        pr3 = pool.tile([P, NB, JP, E], F32, tag="pr3")
        e_pr3 = PL if cfg.get("pr3_eng", "dve") == "pool" else V
        e_pr3.tensor_tensor(
            pr3[:], eq3[:], sc[:].unsqueeze(2).to_broadcast([P, NB, JP, E]),
            op=Alu.mult)
        V.tensor_reduce(gat[:, :, GD:TOPK], pr3[:], axis=AxX, op=Alu.add)

    # weights = 2.5 * gat / sum(gat)
    s1 = pool.tile([P, NB], F32, tag="s1")
    V.tensor_reduce(s1[:], gat[:], axis=AxX, op=Alu.add)
    r1 = pool.tile([P, NB], F32, tag="r1")
    V.reciprocal(r1[:], s1[:])
    r25 = pool.tile([P, NB], F32, tag="r25")
    V.tensor_scalar(out=r25[:], in0=r1[:], scalar1=float(ROUTE_SCALE),
                    scalar2=None, op0=Alu.mult)
    e_wo.tensor_tensor(
        wi[:, :, 0:TOPK], gat[:],
        r25[:].unsqueeze(2).to_broadcast([P, NB, 8]), op=Alu.mult)

    wv = wi_out[t0:t0 + NB * 128, :].rearrange("(b p) j -> p b j", p=128)
    return wv, wi


def _body(nc, pools, dram, cfg):
    cpool, xpool, wpool, scpool, psA, psB = pools
    xp, wi_out, whl_sb, w8_sb, br8, id_sb = dram
    mode = cfg.get("mode", "full")

    NBT = cfg.get("nbt", 2)          # chunks per top-k batch
    NBC = CHUNK // 128
    # batch schedule: list of chunk-counts; optional smaller tail batches
    sched = []
    n = NCHUNK
    tail1 = cfg.get("tail1", 2)      # how many final chunks run as singles
    while n > tail1:
        take = min(NBT, n - tail1)
        sched.append(take)
        n -= take
    sched.extend([1] * n)
    chunk_batch = []                 # chunk -> (batch_idx, offset_chunks, size)
    for bi, sz in enumerate(sched):
        for off in range(sz):
            chunk_batch.append((bi, off, sz))

    state = {"sc": None, "prev": None, "outq": []}
    OUT_LAG = cfg.get("out_lag", 2)   # batches to hold before SP out-DMA

    def flush_outq(keep):
        while len(state["outq"]) > keep:
            wv, wi = state["outq"].pop(0)
            nc.sync.dma_start(wv, wi[:])

    def flush_prev():
        # emit transpose + sigmoid (+ top-k at pair end) for the previous
        # chunk; called after the next chunk's main matmuls so the PE never
        # stalls on the combine.
        if state["prev"] is None:
            return
        pc, lg, elo, pb8 = state["prev"]
        state["prev"] = None
        bi, off, bsz = chunk_batch[pc]
        NBb = bsz * NBC
        streams = cfg.get("streams", "both")
        pt = psB.tile([128, NBC, E], F32, tag="pt")
        for j in range(NBC):
            js = slice(j * 128, (j + 1) * 128)
            mms = []
            if streams in ("both", "hi"):
                mms.append(lg[0:E, js])
                mms.append(elo[:, js])
            if streams in ("both", "lo"):
                mms.append(pb8[:, js])
            for i, src in enumerate(mms):
                nc.tensor.matmul(pt[:, j, :], src, id_sb[0:E, :],
                                 is_transpose=True,
                                 start=(i == 0), stop=(i == len(mms) - 1))
        if off == 0:
            sc_new = scpool.tile([128, NBb, E], F32, tag="sc")
            state["sc"] = sc_new
        sc = state["sc"]
        nc.scalar.activation(sc[:, off * NBC:(off + 1) * NBC, :],
                             pt[:], Act.Sigmoid)
        if mode == "mm":
            nc.sync.dma_start(wi_out[pc * CHUNK:pc * CHUNK + 128, 0:8],
                              sc[:, 0, 0:8])
            return
        if off == bsz - 1:
            out = _topk_batch(nc, wpool, sc, br8, wi_out,
                              (pc - bsz + 1) * CHUNK, cfg, NBb)
            state["outq"].append(out)

    KTB = cfg.get("ktb", 4)          # k-tiles per x DMA
    PB = cfg.get("pb", 2)            # chunks batched per DMA/compute group
    xks = {}
    for c in range(NCHUNK):
        flush_outq(OUT_LAG)
        if c % PB == 0:
            for cc in range(c, min(c + PB, NCHUNK)):
                xk_new = []
                for k0 in range(0, KT, KTB):
                    tl = xpool.tile([KP, KTB, 1536], U8, tag="xt")
                    nc.sync.dma_start(
                        tl[:],
                        xp[k0 * KP:(k0 + KTB) * KP, cc, :].rearrange(
                            "(kb p) b -> p kb b", p=KP))
                    for i in range(KTB):
                        xk_new.append(tl[:, i, :])
                xks[cc] = xk_new
        if mode == "dma":
            xk = xks.pop(c)
            zz = wpool.tile([KP, 1], F32, tag="zz")
            nc.vector.tensor_reduce(zz[:], xk[0][:, 0:8].bitcast(F16)[:, 0:4],
                                    axis=AxX, op=Alu.max)
            continue

        # hi stream: [wh|wl] fp16 stationary -> PA[0:64]=wh.xh, PA[64:]=wl.xh
        # lo stream: w8 e3m4 stationary, x8 residual -> PB = (w*2^7).(res*2^12)
        # All chunks of the DMA group stream through each stationary before
        # it is swapped, halving the Ldweights rate on the PE.
        streams = cfg.get("streams", "both")
        if c % PB == 0:
            group = [cc for cc in range(c, min(c + PB, NCHUNK))]
            pss = {}
            for cc in group:
                ps_n = psA.tile([2 * E, CHUNK], F32, tag="pa")
                pb_n = psA.tile([E, CHUNK], F32, tag="pb")
                pss[cc] = (ps_n, pb_n)
            for k in range(KT):
                if streams in ("both", "hi"):
                    for cc in group:
                        nc.tensor.matmul(
                            pss[cc][0][:],
                            whl_sb[:, k * 2 * E:(k + 1) * 2 * E],
                            xks[cc][k][:, 0:1024].bitcast(F16),
                            start=(k == 0), stop=(k == KT - 1))
                if streams in ("both", "lo"):
                    for cc in group:
                        nc.tensor.matmul(
                            pss[cc][1][:], w8_sb[:, k * E:(k + 1) * E],
                            xks[cc][k][:, 1024:1536].bitcast(F8),
                            start=(k == 0), stop=(k == KT - 1))
            state["pss"] = pss
        xks.pop(c)
        ps, pb = state["pss"].pop(c)

        # evacuate PSUM on ACT (outside the DVE/Pool top-k chains, so the
        # accumulation banks free without queueing behind them); the 2^-19
        # residual scale folds into the pb copy, the final add into the
        # accumulating transposes
        streams = cfg.get("streams", "both")
        lg = wpool.tile([2 * E, CHUNK], F32, tag="lg")
        elo = wpool.tile([E, CHUNK], F32, tag="elo")
        if streams in ("both", "hi"):
            nc.scalar.copy(lg[:], ps[:])
            # wl half sits on partitions 64:128; only a DMA can shift it down
            # to base 0 where the PE transpose can read it
            nc.scalar.dma_start(elo[:], lg[E:2 * E, :])
        pb8 = wpool.tile([E, CHUNK], F32, tag="pb8")
        if streams in ("both", "lo"):
            nc.scalar.mul(pb8[:], pb[:], float(C8))
        flush_prev()
        state["prev"] = (c, lg, elo, pb8)

    flush_prev()
    flush_outq(0)


def _build_nc(n_repeat=1, **cfg):
    import contextlib
    nc = bacc.Bacc(None, target_bir_lowering=False, debug=False)

    xp = nc.declare_dram_parameter("xp", [KT * KP, NCHUNK, 1536], U8,
                                   isOutput=False)
    whl = nc.declare_dram_parameter("whl", [KT * KP, 2 * E], F16,
                                    isOutput=False)
    w8 = nc.declare_dram_parameter("w8", [KT * KP, E], F8, isOutput=False)
    br = nc.declare_dram_parameter("br", [128, E], F32, isOutput=False)
    idn = nc.declare_dram_parameter("idn", [2 * E, E], F32, isOutput=False)
    wi_out = nc.declare_dram_parameter("wi_out", [TPC, 2 * TOPK], F32,
                                       isOutput=True)

    NB = cfg.get("nbt", 2) * (CHUNK // 128)

    with TileContext(nc) as tc:
        with (
            tc.tile_pool(name="const", bufs=1) as cpool,
            tc.tile_pool(name="xts", bufs=cfg.get("xbufs", 12)) as xpool,
            tc.tile_pool(name="work", bufs=cfg.get("wbufs", 2)) as wpool,
            tc.tile_pool(name="scp", bufs=cfg.get("scbufs", 3)) as scpool,
            tc.tile_pool(name="psmm", bufs=cfg.get("psa", 3),
                         space="PSUM") as psA,
            tc.tile_pool(name="pstr", bufs=cfg.get("psb", 2),
                         space="PSUM") as psB,
        ):
            whl_sb = cpool.tile([KP, KT * 2 * E], F16)
            nc.sync.dma_start(
                whl_sb[:].rearrange("p (k e) -> p k e", k=KT),
                whl[:, :].rearrange("(k p) e -> p k e", p=KP))
            w8_sb = cpool.tile([KP, KT * E], F8)
            nc.sync.dma_start(
                w8_sb[:].rearrange("p (k e) -> p k e", k=KT),
                w8[:, :].rearrange("(k p) e -> p k e", p=KP))
            br_sb = cpool.tile([128, E], F32)
            nc.sync.dma_start(br_sb[:], br[:, :])
            id_sb = cpool.tile([2 * E, E], F32)
            nc.sync.dma_start(id_sb[:], idn[:, :])
            br8 = cpool.tile([128, NB, E], F32)
            for b in range(NB):
                nc.vector.tensor_copy(br8[:, b, :], br_sb[:])

            pools = (cpool, xpool, wpool, scpool, psA, psB)
            dram = (xp, wi_out, whl_sb, w8_sb, br8, id_sb)
            rep_ctx = tc.For_i(0, n_repeat, 1) if n_repeat > 1 \
                else contextlib.nullcontext()
            with rep_ctx:
                for _ in range(cfg.get("unroll", 1)):
                    _body(nc, pools, dram, cfg)

    nc.compile()
    return nc


def _get_nc():
    if "nc" not in _CACHE:
        _CACHE["nc"] = _build_nc()
    return _CACHE["nc"]


def _prep_inputs(x, weight, bias, **_unused):
    x = np.asarray(x, dtype=np.float32)
    weight = np.asarray(weight, dtype=np.float32)
    bias = np.asarray(bias, dtype=np.float32)
    assert x.shape == (T, DIM) and weight.shape == (E, DIM - 1)

    br = np.tile(bias[None, :], (128, 1)).astype(np.float32)
    idn = np.vstack([np.eye(E, dtype=np.float32),
                     np.eye(E, dtype=np.float32)])

    wt = np.zeros((KT * KP, E), dtype=np.float32)
    wt[:DIM - 1] = weight.T
    whl = np.empty((KT * KP, 2 * E), dtype=np.float16)
    whl[:, :E] = wt
    whl[:, E:] = wt - whl[:, :E].astype(np.float32)
    w8 = (wt * KW8).astype(ml_dtypes.float8_e3m4)

    in_maps = []
    for c in range(NCORES):
        xtc = np.zeros((KT * KP, TPC), dtype=np.float32)
        xtc[:DIM - 1] = x[c * TPC:(c + 1) * TPC, 1:].T
        xh = xtc.astype(np.float16)
        res = (xtc - xh.astype(np.float32)) * np.float32(K8)
        x8 = res.astype(ml_dtypes.float8_e3m4)
        xh_u8 = xh.reshape(KT * KP, NCHUNK, CHUNK).view(np.uint8)
        x8_u8 = x8.reshape(KT * KP, NCHUNK, CHUNK).view(np.uint8)
        xpk = np.concatenate([xh_u8, x8_u8], axis=2)
        in_maps.append({"xp": xpk, "whl": whl, "w8": w8, "br": br,
                        "idn": idn})
    return in_maps


def kernel(x, weight, bias):
    nc = _get_nc()
    in_maps = _prep_inputs(x, weight, bias)
    out = run_bass_kernel_spmd(nc, in_maps, list(range(NCORES)))
    _CACHE["last_result"] = out
    res = out.results
    wi = np.concatenate([res[c]["wi_out"] for c in range(NCORES)], axis=0)
    weights = np.ascontiguousarray(wi[:, 0:TOPK])
    indices = np.ascontiguousarray(wi[:, TOPK:2 * TOPK]).view(np.int32)
    return weights, indices


# ---------------------------------------------------------------------------
# benchmarking helpers (not used by the grader; kernel() above is the entry)
# ---------------------------------------------------------------------------

def _timed_runner(nc, in_maps):
    """Mirror bass2jax.run_bass_via_pjrt's multi-core path, but keep inputs
    resident on device and return a closure that runs + blocks."""
    import jax
    from jax.sharding import Mesh, PartitionSpec, NamedSharding
    from jax.experimental.shard_map import shard_map
    from concourse import bass2jax

    bass2jax.install_neuronx_cc_hook()
    if nc.dbg_addr is not None:
        in_maps = [
            {**m, nc.dbg_addr.name: np.zeros((1, 2), np.uint32)} for m in in_maps
        ]
    partition_name = nc.partition_id_tensor.name if nc.partition_id_tensor else None
    in_names, out_names, out_avals, zero_outs = [], [], [], []
    for alloc in nc.m.functions[0].allocations:
        if not isinstance(alloc, mybir.MemoryLocationSet):
            continue
        name = alloc.memorylocations[0].name
        if alloc.kind == "ExternalInput":
            if name != partition_name:
                in_names.append(name)
        elif alloc.kind == "ExternalOutput":
            shape = tuple(alloc.tensor_shape)
            dtype = mybir.dt.np(alloc.dtype)
            out_names.append(name)
            out_avals.append(jax.core.ShapedArray(shape, dtype))
            zero_outs.append(np.zeros(shape, dtype))
    n_params = len(in_names)
    n_cores = len(in_maps)
    all_in_names = list(in_names) + list(out_names)
    if partition_name is not None:
        all_in_names.append(partition_name)

    def _b(*args):
        operands = list(args)
        if partition_name is not None:
            operands.append(bass2jax.partition_id_tensor())
        outs = bass2jax._bass_exec_p.bind(
            *operands,
            out_avals=tuple(out_avals),
            in_names=tuple(all_in_names),
            out_names=tuple(out_names),
            lowering_input_output_aliases=(),
            sim_require_finite=True,
            sim_require_nnan=True,
            nc=nc,
        )
        return tuple(outs)

    devices = jax.devices()[:n_cores]
    mesh = Mesh(np.asarray(devices), ("core",))
    in_specs = (PartitionSpec("core"),) * (n_params + len(out_names))
    out_specs = (PartitionSpec("core"),) * len(out_names)
    fn = jax.jit(shard_map(_b, mesh=mesh, in_specs=in_specs,
                           out_specs=out_specs, check_rep=False))
    sh = NamedSharding(mesh, PartitionSpec("core"))
    concat_in = [
        jax.device_put(
            np.concatenate([np.asarray(in_maps[c][nm]) for c in range(n_cores)], 0),
            sh)
        for nm in in_names
    ]
    concat_zeros = [
        jax.device_put(np.zeros((n_cores * z.shape[0], *z.shape[1:]), z.dtype), sh)
        for z in zero_outs
    ]

    def run():
        outs = fn(*concat_in, *concat_zeros)
        jax.block_until_ready(outs)
        return outs

    return run


def bench_nc(nc_r, nc_1, in_maps, n_repeat, trials=16):
    import time
    run_r = _timed_runner(nc_r, in_maps)
    run_1 = _timed_runner(nc_1, in_maps)
    run_r(); run_1()
    ts_r, ts_1, deltas = [], [], []
    for _ in range(trials):
        t0 = time.perf_counter(); run_1(); t1 = time.perf_counter()
        run_r(); t2 = time.perf_counter()
        ts_1.append(t1 - t0); ts_r.append(t2 - t1)
        deltas.append((t2 - t1) - (t1 - t0))
    for tag, ts in ((n_repeat, ts_r), (1, ts_1)):
        print(f"    repeat={tag:3d}: min {min(ts)*1e3:8.3f} ms  "
              f"med {sorted(ts)[len(ts)//2]*1e3:8.3f} ms")
    dmin = min(ts_r) - min(ts_1)
    dmed = sorted(deltas)[len(deltas)//2]
    print(f"    delta: min-based {dmin*1e3:7.3f} ms   "
          f"median-paired {dmed*1e3:7.3f} ms")
    # guard against machine drift: the unpaired min-difference can go
    # negative when interference hits the short run; fall back to the
    # median paired delta which is robust to slow drift
    cands = [d for d in (dmin, dmed) if d > 0]
    est = min(cands) if cands else abs(dmed)
    return est / (n_repeat - 1) * 1e9  # per-iteration


def bench(x, weight, bias, n_repeat=256, trials=28, **cfg):
    u = cfg.get("unroll", 1)
    n_repeat = n_repeat // u
    in_maps = _prep_inputs(x, weight, bias)
    key = tuple(sorted(cfg.items()))
    if ("ncr", key) not in _CACHE:
        _CACHE[("ncr", key)] = _build_nc(n_repeat, **cfg)
        _CACHE[("nc1", key)] = _build_nc(1, **cfg)
    per_iter = bench_nc(_CACHE[("ncr", key)], _CACHE[("nc1", key)],
                        in_maps, n_repeat, trials)
    return per_iter / u



# revision 5
# speedup vs baseline: 1.0567x; 1.0567x over previous
"""MoE gate (group-limited greedy routing) on 8 Trainium2 NeuronCores.

Math (per token t):
    logits = x[t, 1:] @ weight.T                    (64 experts)
    scores = sigmoid(logits)
    sb     = scores + bias
    group_scores[g] = sum(top2(sb[g*8:(g+1)*8]))    (8 groups)
    keep top-4 groups; mask the rest to -inf
    top-8 experts of masked sb -> indices
    weights = 2.5 * normalize(scores[indices])

Device strategy per core (4096 tokens), v4:
  - x streamed at 3 bytes/elem: fp16 hi + e4m3 fp8 of the scaled residual
    (res * 2^12), packed per (k-pair, chunk) into one u8 DMA of 3072B/row.
    W is fp16 hi/lo packed [wh|wl] (fp32-exact); the residual runs as
    fp8e4 DoubleRow matmuls (0.5 cyc/row) with w8 = e4m3(w * 2^7).
    logits = (PA_hi + PA_lo) + 2^-19 * PB.
  - fp32 transpose (PE identity matmul) to token-major, sigmoid on ACT.
  - top-k via exact mantissa packing: z = sc + bias + 2.1 lies in [2,4),
    key = (mant(z) << 6) | tag, tag = 63-expert_id (list A) / 6-bit
    quantized bias (list B); one native DVE top-8 Max per list gives
    sorted values + ids + dequantizable scores.  Only bitwise/shift ALU
    ops touch the >24-bit keys (arith ops round through fp32).
"""

import sys

sys.path.insert(0, "/opt/trn_rl_repo")

import numpy as np
import ml_dtypes
import concourse.bacc as bacc
import concourse.mybir as mybir
from concourse.tile import TileContext
from concourse.bass_utils import run_bass_kernel_spmd

F32 = mybir.dt.float32
F16 = mybir.dt.float16
F8E4 = mybir.dt.float8e4
U8 = mybir.dt.uint8
I32 = mybir.dt.int32
Alu = mybir.AluOpType
Act = mybir.ActivationFunctionType
AxX = mybir.AxisListType.X
DRMODE = mybir.MatmulPerfMode.DoubleRow

T = 32768
DIM = 2048
E = 64
G = 8
GS = E // G
TOPK = 8
ROUTE_SCALE = 2.5

NCORES = 8
TPC = T // NCORES    # 4096 tokens per core
CHUNK = 512
NCHUNK = TPC // CHUNK
KP = 128
KT = DIM // KP       # 16 k-tiles
PAIRS = KT // 2      # 8 DoubleRow k-pairs
ROWB = 3072          # bytes per (pair, partition, chunk): 2KB fp16 + 1KB fp8

K8 = 4096.0          # 2^12 residual scale
KW8 = 128.0          # 2^7 w scale
C8 = 1.0 / (K8 * KW8)

OFF = 2.1            # forces z = sc + bias + OFF into [2, 4)
NEG = -1.0e9

_CACHE = {}


def _topk_batch(nc, pool, sg, consts, wi_out, t0, cfg, NB):
    """Group-limited exact top-8 for one [128, NB, 64] sigmoid batch."""
    P = 128
    V = nc.vector
    PL = nc.gpsimd
    br8, idrow, bqrow, scal = consts

    def pick(key, default="dve"):
        return PL if cfg.get(key, default) == "pool" else V

    def t4(ap):
        return ap.rearrange("p b (g s) -> p b g s", s=GS)

    # z = sigmoid + bias + OFF  (fp32; z in (2, 4))
    z = pool.tile([P, NB, E], F32, tag="z")
    pick("z_eng", "pool").tensor_tensor(z[:], sg[:], br8[:, 0:NB, :],
                                        op=Alu.add)

    # group top-2 sum: m1 = group max, m2 = max after masking m1 out
    m1 = pool.tile([P, NB, G], F32, tag="m1")
    pick("m1_eng").tensor_reduce(m1[:], t4(z[:]), axis=AxX, op=Alu.max)
    eqg = pool.tile([P, NB, E], F32, tag="eqg")
    pick("eq_eng").tensor_tensor(
        t4(eqg[:]), t4(z[:]),
        m1[:].unsqueeze(3).to_broadcast([P, NB, G, GS]), op=Alu.is_equal)
    z2 = pool.tile([P, NB, E], F32, tag="z2")
    V.scalar_tensor_tensor(out=z2[:], in0=eqg[:], scalar=NEG, in1=z[:],
                           op0=Alu.mult, op1=Alu.add)
    m2 = pool.tile([P, NB, G], F32, tag="m2")
    pick("m2_eng").tensor_reduce(m2[:], t4(z2[:]), axis=AxX, op=Alu.max)
    gst = pool.tile([P, NB, G], F32, tag="gst")
    pick("gs_eng", "pool").tensor_tensor(gst[:], m1[:], m2[:], op=Alu.add)

    # top-4 groups: threshold at 4th largest group score
    g8 = pool.tile([P, NB, 8], F32, tag="g8")
    for b in range(NB):
        V.max(out=g8[:, b, :], in_=gst[:, b, :])
    pen = pool.tile([P, NB, G], F32, tag="pen")
    V.tensor_tensor(pen[:], gst[:],
                    g8[:, :, 3:4].to_broadcast([P, NB, G]), op=Alu.is_lt)
    # pen: 1.0f -> 0x80000000 (sign-bit mask), 0.0f -> 0
    penM = pool.tile([P, NB, G], I32, tag="penM")
    V.tensor_scalar(out=penM[:], in0=pen[:].bitcast(I32), scalar1=8,
                    scalar2=None, op0=Alu.logical_shift_left)

    # exact packing: key = (mant(z) << 6) | tag  (bitwise ops only)
    keym = pool.tile([P, NB, E], I32, tag="keym")
    V.tensor_scalar(out=keym[:], in0=z[:].bitcast(I32), scalar1=0x7FFFFF,
                    scalar2=None, op0=Alu.bitwise_and)
    V.tensor_scalar(out=keym[:], in0=keym[:], scalar1=6, scalar2=None,
                    op0=Alu.logical_shift_left)
    mkA = pool.tile([P, NB, E], I32, tag="mkA")
    eka = pick("ka_eng")
    eka.tensor_tensor(mkA[:], keym[:],
                      idrow[:].unsqueeze(1).to_broadcast([P, NB, E]),
                      op=Alu.bitwise_or)
    eka.tensor_tensor(
        t4(mkA[:]), t4(mkA[:]),
        penM[:].unsqueeze(3).to_broadcast([P, NB, G, GS]), op=Alu.bitwise_or)
    mkB = pool.tile([P, NB, E], I32, tag="mkB")
    ekb = pick("kb_eng")
    ekb.tensor_tensor(mkB[:], keym[:],
                      bqrow[:].unsqueeze(1).to_broadcast([P, NB, E]),
                      op=Alu.bitwise_or)
    ekb.tensor_tensor(
        t4(mkB[:]), t4(mkB[:]),
        penM[:].unsqueeze(3).to_broadcast([P, NB, G, GS]), op=Alu.bitwise_or)

    # native top-8 per block (descending; float-bit order == int order here)
    va = pool.tile([P, NB, 8], F32, tag="va")
    vb = pool.tile([P, NB, 8], F32, tag="vb")
    for b in range(NB):
        V.max(out=va[:, b, :], in_=mkA[:, b, :].bitcast(F32))
        V.max(out=vb[:, b, :], in_=mkB[:, b, :].bitcast(F32))

    # wi packs [weights f32 | indices i32]
    wi = pool.tile([P, NB, 2 * TOPK], F32, tag="wi")
    ixv = wi[:, :, TOPK:2 * TOPK].bitcast(I32)
    ia = pool.tile([P, NB, 8], I32, tag="ia")
    V.tensor_scalar(out=ia[:], in0=va[:].bitcast(I32), scalar1=63,
                    scalar2=None, op0=Alu.bitwise_and)
    V.tensor_scalar(out=ixv, in0=ia[:], scalar1=63, scalar2=None,
                    op0=Alu.bitwise_xor)

    # reconstruct selected scores: z = bits((v>>6) + 0x40000000),
    # sc = (z - OFF - bmin) - bq * step
    bqi = pool.tile([P, NB, 8], I32, tag="bqi")
    V.tensor_scalar(out=bqi[:], in0=vb[:].bitcast(I32), scalar1=63,
                    scalar2=None, op0=Alu.bitwise_and)
    bqf = pool.tile([P, NB, 8], F32, tag="bqf")
    V.tensor_copy(bqf[:], bqi[:])
    zm = pool.tile([P, NB, 8], I32, tag="zm")
    V.tensor_scalar(out=zm[:], in0=vb[:].bitcast(I32), scalar1=6,
                    scalar2=None, op0=Alu.logical_shift_right)
    V.tensor_scalar(out=zm[:], in0=zm[:], scalar1=0x40000000, scalar2=None,
                    op0=Alu.add)
    stp = pool.tile([P, NB, 8], F32, tag="stp")
    V.tensor_scalar(out=stp[:], in0=bqf[:], scalar1=scal[:, 0:1],
                    scalar2=None, op0=Alu.mult)
    sc = pool.tile([P, NB, 8], F32, tag="sc8")
    V.scalar_tensor_tensor(out=sc[:], in0=zm[:].bitcast(F32),
                           scalar=scal[:, 1:2], in1=stp[:],
                           op0=Alu.subtract, op1=Alu.subtract)

    # weights = 2.5 * sc / sum(sc)
    s1 = pool.tile([P, NB], F32, tag="s1")
    V.tensor_reduce(s1[:], sc[:], axis=AxX, op=Alu.add)
    r1 = pool.tile([P, NB], F32, tag="r1")
    V.reciprocal(r1[:], s1[:])
    r25 = pool.tile([P, NB], F32, tag="r25")
    V.tensor_scalar(out=r25[:], in0=r1[:], scalar1=float(ROUTE_SCALE),
                    scalar2=None, op0=Alu.mult)
    pick("wo_eng").tensor_tensor(
        wi[:, :, 0:TOPK], sc[:],
        r25[:].unsqueeze(2).to_broadcast([P, NB, 8]), op=Alu.mult)

    wv = wi_out[t0:t0 + NB * 128, :].rearrange("(b p) j -> p b j", p=128)
    return wv, wi


def _body(nc, pools, dram, cfg):
    cpool, xpool, wpool, scpool, psA, psB = pools
    xp, wi_out, whl_sb, w8_sb, consts, id_sb = dram
    mode = cfg.get("mode", "full")

    NBT = cfg.get("nbt", 2)
    NBC = CHUNK // 128
    sched = []
    n = NCHUNK
    tail1 = cfg.get("tail1", 2)
    while n > tail1:
        take = min(NBT, n - tail1)
        sched.append(take)
        n -= take
    sched.extend([1] * n)
    chunk_batch = []
    for bi, sz in enumerate(sched):
        for off in range(sz):
            chunk_batch.append((bi, off, sz))

    state = {"sg": None, "prev": None, "outq": []}
    OUT_LAG = cfg.get("out_lag", 2)

    def flush_outq(keep):
        while len(state["outq"]) > keep:
            wv, wi = state["outq"].pop(0)
            nc.sync.dma_start(wv, wi[:])

    def flush_prev():
        if state["prev"] is None:
            return
        pc, lg, el2 = state["prev"]
        state["prev"] = None
        bi, off, bsz = chunk_batch[pc]
        NBb = bsz * NBC
        pt = psB.tile([128, NBC, E], F32, tag="pt")
        for j in range(NBC):
            js = slice(j * 128, (j + 1) * 128)
            nc.tensor.matmul(pt[:, j, :], lg[0:E, js], id_sb[0:E, :],
                             is_transpose=True, start=True, stop=False)
            nc.tensor.matmul(pt[:, j, :], el2[:, js], id_sb[0:E, :],
                             is_transpose=True, start=False, stop=True)
        if off == 0:
            sg_new = scpool.tile([128, NBb, E], F32, tag="sg")
            state["sg"] = sg_new
        sg = state["sg"]
        nc.scalar.activation(sg[:, off * NBC:(off + 1) * NBC, :],
                             pt[:], Act.Sigmoid)
        if mode == "mm":
            nc.sync.dma_start(wi_out[pc * CHUNK:pc * CHUNK + 128, 0:8],
                              sg[:, 0, 0:8])
            return
        if off == bsz - 1:
            out = _topk_batch(nc, wpool, sg, consts, wi_out,
                              (pc - bsz + 1) * CHUNK, cfg, NBb)
            state["outq"].append(out)

    KTB = cfg.get("ktb", 2)          # k-pairs per x DMA
    PB = cfg.get("pb", 2)            # chunks batched per DMA/compute group
    xks = {}
    for c in range(NCHUNK):
        flush_outq(OUT_LAG)
        if c % PB == 0:
            for cc in range(c, min(c + PB, NCHUNK)):
                tls = []
                for t0 in range(0, PAIRS, KTB):
                    tl = xpool.tile([KP, KTB, ROWB], U8, tag="xt")
                    nc.sync.dma_start(
                        tl[:],
                        xp[t0 * KP:(t0 + KTB) * KP, cc, :].rearrange(
                            "(tb p) b -> p tb b", p=KP))
                    for i in range(KTB):
                        tls.append(tl[:, i, :])
                xks[cc] = tls
        if mode == "dma":
            xk = xks.pop(c)
            zz = wpool.tile([KP, 1], F32, tag="zz")
            nc.vector.tensor_reduce(zz[:], xk[0][:, 0:8].bitcast(F16)[:, 0:4],
                                    axis=AxX, op=Alu.max)
            continue

        if c % PB == 0:
            group = [cc for cc in range(c, min(c + PB, NCHUNK))]
            pss = {}
            for cc in group:
                pa = psA.tile([2 * E, CHUNK], F32, tag="pa")
                pb = psA.tile([E, CHUNK], F32, tag="pb")
                pss[cc] = (pa, pb)
            # hi stream: [wh|wl] fp16 stationary, all chunks per stationary
            for k in range(KT):
                t, s = k // 2, k % 2
                for cc in group:
                    nc.tensor.matmul(
                        pss[cc][0][:],
                        whl_sb[:, k * 2 * E:(k + 1) * 2 * E],
                        xks[cc][t][:, s * 1024:(s + 1) * 1024].bitcast(F16),
                        start=(k == 0), stop=(k == KT - 1))
            # lo stream: e4m3 DoubleRow, one matmul per k-pair
            for t in range(PAIRS):
                for cc in group:
                    nc.tensor.matmul(
                        pss[cc][1][:],
                        w8_sb[:, t, :, :],
                        xks[cc][t][:, 2048:3072].bitcast(F8E4).rearrange(
                            "p (s c) -> p s c", s=2),
                        start=(t == 0), stop=(t == PAIRS - 1),
                        perf_mode=DRMODE)
            state["pss"] = pss
        xks.pop(c)
        pa, pb = state["pss"].pop(c)

        # evacuate: lg = pa (ACT); elo = lg[64:128] via DMA shift;
        # el2 = pb * C8 + elo (frees both PSUM banks)
        lg = wpool.tile([2 * E, CHUNK], F32, tag="lg")
        nc.scalar.copy(lg[:], pa[:])
        elo = wpool.tile([E, CHUNK], F32, tag="elo")
        nc.scalar.dma_start(elo[:], lg[E:2 * E, :])
        el2 = wpool.tile([E, CHUNK], F32, tag="el2")
        eel = nc.gpsimd if cfg.get("el2_eng", "dve") == "pool" else nc.vector
        eel.scalar_tensor_tensor(out=el2[:], in0=pb[:], scalar=float(C8),
                                 in1=elo[:], op0=Alu.mult, op1=Alu.add)
        flush_prev()
        state["prev"] = (c, lg, el2)

    flush_prev()
    flush_outq(0)


def _build_nc(n_repeat=1, **cfg):
    import contextlib
    nc = bacc.Bacc(None, target_bir_lowering=False, debug=False)

    xp = nc.declare_dram_parameter("xp", [PAIRS * KP, NCHUNK, ROWB], U8,
                                   isOutput=False)
    whl = nc.declare_dram_parameter("whl", [KT * KP, 2 * E], F16,
                                    isOutput=False)
    w8 = nc.declare_dram_parameter("w8", [KP, PAIRS, 2, E], F8E4,
                                   isOutput=False)
    br = nc.declare_dram_parameter("br", [128, E], F32, isOutput=False)
    idr = nc.declare_dram_parameter("idr", [128, E], I32, isOutput=False)
    bqr = nc.declare_dram_parameter("bqr", [128, E], I32, isOutput=False)
    scl = nc.declare_dram_parameter("scl", [128, 2], F32, isOutput=False)
    idn = nc.declare_dram_parameter("idn", [E, E], F32, isOutput=False)
    wi_out = nc.declare_dram_parameter("wi_out", [TPC, 2 * TOPK], F32,
                                       isOutput=True)

    NB = cfg.get("nbt", 2) * (CHUNK // 128)

    with TileContext(nc) as tc:
        with (
            tc.tile_pool(name="const", bufs=1) as cpool,
            tc.tile_pool(name="xts", bufs=cfg.get("xbufs", 12)) as xpool,
            tc.tile_pool(name="work", bufs=cfg.get("wbufs", 2)) as wpool,
            tc.tile_pool(name="scp", bufs=cfg.get("scbufs", 3)) as scpool,
            tc.tile_pool(name="psmm", bufs=cfg.get("psa", 3),
                         space="PSUM") as psA,
            tc.tile_pool(name="pstr", bufs=cfg.get("psb", 2),
                         space="PSUM") as psB,
        ):
            whl_sb = cpool.tile([KP, KT * 2 * E], F16)
            nc.sync.dma_start(
                whl_sb[:].rearrange("p (k e) -> p k e", k=KT),
                whl[:, :].rearrange("(k p) e -> p k e", p=KP))
            w8_sb = cpool.tile([KP, PAIRS, 2, E], F8E4)
            nc.sync.dma_start(w8_sb[:], w8[:, :, :, :])
            br_sb = cpool.tile([128, E], F32)
            nc.sync.dma_start(br_sb[:], br[:, :])
            idr_sb = cpool.tile([128, E], I32)
            nc.sync.dma_start(idr_sb[:], idr[:, :])
            bqr_sb = cpool.tile([128, E], I32)
            nc.sync.dma_start(bqr_sb[:], bqr[:, :])
            scl_sb = cpool.tile([128, 2], F32)
            nc.sync.dma_start(scl_sb[:], scl[:, :])
            id_sb = cpool.tile([E, E], F32)
            nc.sync.dma_start(id_sb[:], idn[:, :])
            br8 = cpool.tile([128, NB, E], F32)
            for b in range(NB):
                nc.vector.tensor_copy(br8[:, b, :], br_sb[:])

            pools = (cpool, xpool, wpool, scpool, psA, psB)
            consts = (br8, idr_sb, bqr_sb, scl_sb)
            dram = (xp, wi_out, whl_sb, w8_sb, consts, id_sb)
            rep_ctx = tc.For_i(0, n_repeat, 1) if n_repeat > 1 \
                else contextlib.nullcontext()
            with rep_ctx:
                for _ in range(cfg.get("unroll", 1)):
                    _body(nc, pools, dram, cfg)

    nc.compile()
    return nc


def _get_nc():
    if "nc" not in _CACHE:
        _CACHE["nc"] = _build_nc()
    return _CACHE["nc"]


def _prep_inputs(x, weight, bias, **_unused):
    x = np.asarray(x, dtype=np.float32)
    weight = np.asarray(weight, dtype=np.float32)
    bias = np.asarray(bias, dtype=np.float32)
    assert x.shape == (T, DIM) and weight.shape == (E, DIM - 1)

    br = np.tile((bias + np.float32(OFF))[None, :], (128, 1)).astype(
        np.float32)
    idrow = np.tile((63 - np.arange(E, dtype=np.int32))[None, :], (128, 1))
    bmin, bmax = float(bias.min()), float(bias.max())
    step = (bmax - bmin) / 63.0 if bmax > bmin else 1.0
    bq = np.clip(np.round((bias - bmin) / step), 0, 63).astype(np.int32)
    bqrow = np.tile(bq[None, :], (128, 1))
    scl = np.tile(np.array([[step, OFF + bmin]], dtype=np.float32), (128, 1))
    idn = np.eye(E, dtype=np.float32)

    wt = np.zeros((KT * KP, E), dtype=np.float32)
    wt[:DIM - 1] = weight.T
    whl = np.empty((KT * KP, 2 * E), dtype=np.float16)
    whl[:, :E] = wt
    whl[:, E:] = wt - whl[:, :E].astype(np.float32)
    # w8[p, t, s, e] = e4m3(w[(2t+s)*128+p, e] * 2^7)
    w8 = (wt * KW8).astype(ml_dtypes.float8_e4m3)
    w8 = w8.reshape(PAIRS, 2, KP, E).transpose(2, 0, 1, 3).copy()

    in_maps = []
    for c in range(NCORES):
        xtc = np.zeros((KT * KP, TPC), dtype=np.float32)
        xtc[:DIM - 1] = x[c * TPC:(c + 1) * TPC, 1:].T
        xh = xtc.astype(np.float16)
        res = (xtc - xh.astype(np.float32)) * np.float32(K8)
        x8 = res.astype(ml_dtypes.float8_e4m3)
        # per (pair, partition, chunk): [xh(2t) 1KB | xh(2t+1) 1KB |
        #                                x8(2t) .5KB | x8(2t+1) .5KB]
        xh_u8 = xh.reshape(PAIRS, 2, KP, NCHUNK, CHUNK).view(np.uint8)
        x8_u8 = x8.reshape(PAIRS, 2, KP, NCHUNK, CHUNK).view(np.uint8)
        xpk = np.empty((PAIRS, KP, NCHUNK, ROWB), dtype=np.uint8)
        xpk[:, :, :, 0:1024] = xh_u8[:, 0]
        xpk[:, :, :, 1024:2048] = xh_u8[:, 1]
        xpk[:, :, :, 2048:2560] = x8_u8[:, 0]
        xpk[:, :, :, 2560:3072] = x8_u8[:, 1]
        xpk = xpk.reshape(PAIRS * KP, NCHUNK, ROWB)
        in_maps.append({"xp": xpk, "whl": whl, "w8": w8, "br": br,
                        "idr": idrow, "bqr": bqrow, "scl": scl, "idn": idn})
    return in_maps


def kernel(x, weight, bias):
    nc = _get_nc()
    in_maps = _prep_inputs(x, weight, bias)
    out = run_bass_kernel_spmd(nc, in_maps, list(range(NCORES)))
    _CACHE["last_result"] = out
    res = out.results
    wi = np.concatenate([res[c]["wi_out"] for c in range(NCORES)], axis=0)
    weights = np.ascontiguousarray(wi[:, 0:TOPK])
    indices = np.ascontiguousarray(wi[:, TOPK:2 * TOPK]).view(np.int32)
    return weights, indices


# ---------------------------------------------------------------------------
# benchmarking helpers (not used by the grader; kernel() above is the entry)
# ---------------------------------------------------------------------------

def _timed_runner(nc, in_maps):
    """Mirror bass2jax.run_bass_via_pjrt's multi-core path, but keep inputs
    resident on device and return a closure that runs + blocks."""
    import jax
    from jax.sharding import Mesh, PartitionSpec, NamedSharding
    from jax.experimental.shard_map import shard_map
    from concourse import bass2jax

    bass2jax.install_neuronx_cc_hook()
    if nc.dbg_addr is not None:
        in_maps = [
            {**m, nc.dbg_addr.name: np.zeros((1, 2), np.uint32)} for m in in_maps
        ]
    partition_name = nc.partition_id_tensor.name if nc.partition_id_tensor else None
    in_names, out_names, out_avals, zero_outs = [], [], [], []
    for alloc in nc.m.functions[0].allocations:
        if not isinstance(alloc, mybir.MemoryLocationSet):
            continue
        name = alloc.memorylocations[0].name
        if alloc.kind == "ExternalInput":
            if name != partition_name:
                in_names.append(name)
        elif alloc.kind == "ExternalOutput":
            shape = tuple(alloc.tensor_shape)
            dtype = mybir.dt.np(alloc.dtype)
            out_names.append(name)
            out_avals.append(jax.core.ShapedArray(shape, dtype))
            zero_outs.append(np.zeros(shape, dtype))
    n_params = len(in_names)
    n_cores = len(in_maps)
    all_in_names = list(in_names) + list(out_names)
    if partition_name is not None:
        all_in_names.append(partition_name)

    def _b(*args):
        operands = list(args)
        if partition_name is not None:
            operands.append(bass2jax.partition_id_tensor())
        outs = bass2jax._bass_exec_p.bind(
            *operands,
            out_avals=tuple(out_avals),
            in_names=tuple(all_in_names),
            out_names=tuple(out_names),
            lowering_input_output_aliases=(),
            sim_require_finite=True,
            sim_require_nnan=True,
            nc=nc,
        )
        return tuple(outs)

    devices = jax.devices()[:n_cores]
    mesh = Mesh(np.asarray(devices), ("core",))
    in_specs = (PartitionSpec("core"),) * (n_params + len(out_names))
    out_specs = (PartitionSpec("core"),) * len(out_names)
    fn = jax.jit(shard_map(_b, mesh=mesh, in_specs=in_specs,
                           out_specs=out_specs, check_rep=False))
    sh = NamedSharding(mesh, PartitionSpec("core"))
    concat_in = [
        jax.device_put(
            np.concatenate([np.asarray(in_maps[c][nm]) for c in range(n_cores)], 0),
            sh)
        for nm in in_names
    ]
    concat_zeros = [
        jax.device_put(np.zeros((n_cores * z.shape[0], *z.shape[1:]), z.dtype), sh)
        for z in zero_outs
    ]

    def run():
        outs = fn(*concat_in, *concat_zeros)
        jax.block_until_ready(outs)
        return outs

    return run


def bench_nc(nc_r, nc_1, in_maps, n_repeat, trials=16):
    import time
    run_r = _timed_runner(nc_r, in_maps)
    run_1 = _timed_runner(nc_1, in_maps)
    run_r(); run_1()
    ts_r, ts_1, deltas = [], [], []
    for _ in range(trials):
        t0 = time.perf_counter(); run_1(); t1 = time.perf_counter()
        run_r(); t2 = time.perf_counter()
        ts_1.append(t1 - t0); ts_r.append(t2 - t1)
        deltas.append((t2 - t1) - (t1 - t0))
    for tag, ts in ((n_repeat, ts_r), (1, ts_1)):
        print(f"    repeat={tag:3d}: min {min(ts)*1e3:8.3f} ms  "
              f"med {sorted(ts)[len(ts)//2]*1e3:8.3f} ms")
    dmin = min(ts_r) - min(ts_1)
    dmed = sorted(deltas)[len(deltas)//2]
    print(f"    delta: min-based {dmin*1e3:7.3f} ms   "
          f"median-paired {dmed*1e3:7.3f} ms")
    cands = [d for d in (dmin, dmed) if d > 0]
    est = min(cands) if cands else abs(dmed)
    return est / (n_repeat - 1) * 1e9


def bench(x, weight, bias, n_repeat=256, trials=28, **cfg):
    u = cfg.get("unroll", 1)
    n_repeat = n_repeat // u
    in_maps = _prep_inputs(x, weight, bias)
    key = tuple(sorted(cfg.items()))
    if ("ncr", key) not in _CACHE:
        _CACHE[("ncr", key)] = _build_nc(n_repeat, **cfg)
        _CACHE[("nc1", key)] = _build_nc(1, **cfg)
    per_iter = bench_nc(_CACHE[("ncr", key)], _CACHE[("nc1", key)],
                        in_maps, n_repeat, trials)
    return per_iter / u
